# revision 40
# baseline (speedup 1.0000x reference)
"""Trainium2 Bass kernel for the vq_codebook problem.

  dist_sq[n,k] = sum_d (x[n,d]-ctrs[k,d])^2 * s[d]
  out = softmax(-dist_sq, axis=1) @ values

Sharding: data-parallel over N (8192 rows of x per core); ctrs/values/s
replicated on all 8 cores. No collectives (forward only).

Math trick: softmax is shift-invariant, so
  softmax(-dist_sq)[n,k] = softmax(2*cross_s[n,k] - c_sq[k])  with
  cross_s = (x*s) @ ctrs.T,  c_sq[k] = sum_d s[d]*ctrs[k,d]^2.
We compute E = exp(2*(cross_s - 0.5*c_sq)) unnormalized (range-checked:
max exponent ~48 < 88, row-max min ~ -27, so fp32 exp never overflows
and denominators stay normal), then
  y[n,:] = (E.T @ values_aug)[n,:256] / (E.T @ values_aug)[n,256]
with values_aug = [values | ones] so the denominator comes from the same
accumulating matmul.

Layouts: x and ctrs are staged TRANSPOSED on the host (layout-only prep,
like the shard slicing itself), so the kernel does zero on-device
transposes. Phase 1 runs transposed (k on partitions, n on free) with an
augmented stationary matrix lhs1 = [[s*ctrs^T], [-0.5*c_sq]]; the moving
operand xsT tiles stream straight from DMA into a 3-deep SBUF ring whose
ones-row (row 64) is written once at startup. Phase 2 uses E chunks as
the stationary operand against values_aug, producing y in natural
[n, d_out] layout.

Scheduling: the loop issues phase1_mm(i), then the DMA for tile i+2,
then phase2(i-1), so the DVE queue (reciprocal+normalize only) and ACT
queue (exp only) never block the PE between tiles. lhs1 is built as 8
separate chunk tiles so phase-1 matmuls start as soon as chunk 0 is
ready. The last tile's write-back is split per 128-row sub-tile to
shorten the serial tail.
"""

import os

os.environ.setdefault("JAX_PLATFORMS", "axon")

import numpy as np

N, D_IN, K, D_OUT = 65536, 64, 1024, 256
NCORES = 8
NS = N // NCORES
TROWS = 512
NTILES = NS // TROWS
KC = K // 128
NSUB = TROWS // 128

USE_F32R = True

_cache = {}


def _build(use_f32r, rows=NS, dma="sync", ph2_bf16=True):
    import concourse.bacc as bacc
    import concourse.tile as tile
    from concourse import mybir

    f32 = mybir.dt.float32
    # fp16 phase-1 operands: same 1 cycle/row as f32r but with a separate,
    # overlappable LDWEIGHTS (f32r matmuls are self-loading, serializing
    # each weight load against the previous stream). e5m10 precision on
    # these O(1) operands matches f32r-class accuracy; the c_sq row is
    # split into main+residual fp16 rows so its quantization error
    # cancels (stationary rows don't add stream time).
    mmdt = mybir.dt.float16 if use_f32r else f32
    p2dt = mybir.dt.bfloat16 if ph2_bf16 else f32
    Exp = mybir.ActivationFunctionType.Exp
    Copy = mybir.ActivationFunctionType.Copy
    NR = D_IN + 2  # 64 data rows + centered c_sq row + const -32 row

    ntiles = rows // TROWS
    nc = bacc.Bacc("TRN2", target_bir_lowering=False, debug=False)
    dma_start = {"sync": nc.sync.dma_start, "gpsimd": nc.gpsimd.dma_start}[dma]
    xT = nc.declare_dram_parameter("xT", [D_IN, rows], f32, isOutput=False)
    ctrsT = nc.declare_dram_parameter("ctrsT", [D_IN, K], f32, isOutput=False)
    values = nc.declare_dram_parameter("values", [K, D_OUT], f32, isOutput=False)
    s = nc.declare_dram_parameter("s", [D_IN], f32, isOutput=False)
    y = nc.declare_dram_parameter("y", [rows, D_OUT], f32, isOutput=True)

    with tile.TileContext(nc) as tc:
        with (
            tc.tile_pool(name="const", bufs=1) as constp,
            tc.tile_pool(name="E", bufs=3) as Ep,
            tc.tile_pool(name="ysb", bufs=3) as yp,
            tc.tile_pool(name="rcp", bufs=8) as rcpp,
            tc.tile_pool(name="psA", bufs=2, space="PSUM") as psA,
            tc.tile_pool(name="psO", bufs=4, space="PSUM") as psO,
        ):
            # Persistent xsT ring; ones row (row 64) written once. DMA lands
            # raw f32 in a staging ring; a single DVE cast rounds to f32r
            # (the matmul dtype requires an explicit rounding producer).
            xsT_ring = [
                constp.tile([NR, TROWS], mmdt, name=f"xsT{r}")
                for r in range(3)
            ]
            xst_ring = [
                constp.tile([D_IN, TROWS], f32, name=f"xst{r}")
                for r in range(3)
            ]
            ones_row = constp.tile([1, TROWS], f32)
            nc.vector.memset(ones_row[:], 1.0)
            # Touch the activation table early so the ~1.3us table load runs
            # during DMA warm-up instead of on the lhs1 critical path. The
            # source comes from a gpsimd memset, which runs earliest.
            act_src = constp.tile([1, 1], f32)
            nc.gpsimd.memset(act_src[:], 0.0)
            act_warm = constp.tile([1, 1], f32)
            nc.scalar.activation(act_warm[:], act_src[:], Exp)
            ones_row2 = constp.tile([2, TROWS], f32)
            nc.vector.memset(ones_row2[:], 1.0)
            for t in xsT_ring:
                nc.vector.tensor_copy(t[D_IN:NR, :], ones_row2[:])
            # Constant -32 row (fp16-exact) that undoes the +32 centering
            # of the c_sq row; placed into row 65 of both lhs1 halves by
            # early DMAs (row 65 is an unaligned partition start).
            neg32_row = constp.tile([1, K // 2], mmdt)
            nc.vector.memset(neg32_row[:], -32.0)

            def phase1_load(i, engine=None):
                n0 = i * TROWS
                start = engine.dma_start if engine is not None else dma_start
                start(xst_ring[i % 3][:], xT[:, n0 : n0 + TROWS])

            def phase1_cast(i):
                xsT = xsT_ring[i % 3]
                nc.vector.tensor_copy(xsT[0:D_IN, :], xst_ring[i % 3][:])
                return xsT

            # Sync-ring FIFO order is the preamble critical path: s (tiny),
            # ctrsT half 0, x tile 0, ctrsT half 1, x tile 1. Tile 0's
            # first phase-1 pairs need only lhs1 half 0 + xsT(0), so the
            # first exp fires as soon as those land.
            s_col = constp.tile([D_IN, 1], f32)
            dma_start(s_col[:], s[:].rearrange("(p o) -> p o", o=1))
            ctrsT_half = [
                constp.tile([D_IN, K // 2], f32, name=f"ctrsT{h}") for h in range(2)
            ]
            dma_start(ctrsT_half[0][:], ctrsT[:, 0 : K // 2])
            phase1_load(0)
            dma_start(ctrsT_half[1][:], ctrsT[:, K // 2 : K])
            phase1_load(1)

            # lhs1 half h (chunks 4h..4h+3): rows 0:64 = s * ctrsT, row 64
            # = -0.5*c_sq + 32 (centered so its fp16 ulp is tiny; the
            # constant is undone by a -64 bias inside the exp, which is
            # softmax-invariant). Chain per half: ACT square -> PE csq
            # matmul -> ACT row64 + scale copy.
            lhs1h = [
                constp.tile([NR, KC // 2, 128], mmdt, name=f"lhs1h{h}")
                for h in range(2)
            ]
            for h in range(2):
                tmp_sq = constp.tile([D_IN, K // 2], f32, name=f"tmpsq{h}")
                nc.scalar.square(tmp_sq[:], ctrsT_half[h][:])
                csq = psO.tile([1, 512], f32, tag="psO")
                nc.tensor.matmul(csq[:], s_col[:], tmp_sq[:])
                nc.scalar.activation(
                    lhs1h[h][D_IN : D_IN + 1, :, :],
                    csq[:].rearrange("o (c k) -> o c k", c=4),
                    Copy,
                    scale=-0.5,
                    bias=32.0,
                )
                nc.scalar.activation(
                    lhs1h[h][0:D_IN, :, :],
                    ctrsT_half[h][:].rearrange("d (c k) -> d c k", c=KC // 2),
                    Copy,
                    scale=s_col[:],
                )
                dma_start(
                    lhs1h[h][D_IN + 1 : NR, :, :],
                    neg32_row[:].rearrange("p (c k) -> p c k", c=4),
                )

            phase1_cast(0)

            vals = constp.tile([128, KC, D_OUT + 2], p2dt)
            vals_stage = constp.tile([128, KC, D_OUT], f32)
            ones_kc = constp.tile([128, KC, 2], f32)

            def vals_dma():
                # Rides the Activation HWDGE ring: by iter 0 the sync ring
                # still has x prefetches queued, and the preamble-critical
                # transfers are already done, so the rings drain in parallel.
                nc.scalar.dma_start(
                    vals_stage[:], values[:].rearrange("(c p) v -> p c v", p=128)
                )
                nc.vector.memset(ones_kc[:], 1.0)

            def vals_copy():
                # Runs at iter 1, after cast(2) on the DVE queue, so the
                # 1MB vals transfer never blocks the x-cast pipeline.
                nc.vector.tensor_copy(vals[:, :, 0:D_OUT], vals_stage[:])
                nc.vector.tensor_copy(vals[:, :, D_OUT : D_OUT + 2], ones_kc[:])

            def phase2_sub(n0, E, a, ysb, split_dma):
                po = psO.tile([128, D_OUT + 2], f32, tag="psO")
                for c in range(KC):
                    nc.tensor.matmul(
                        po[:],
                        E[:, c, a * 128 : (a + 1) * 128],
                        vals[:, c, :],
                        start=(c == 0),
                        stop=(c == KC - 1),
                    )
                rcp = rcpp.tile([128, 1], f32)
                nc.vector.reciprocal(rcp[:], po[:, D_OUT : D_OUT + 1])
                nc.vector.tensor_scalar_mul(ysb[:, a, :], po[:, 0:D_OUT], rcp[:])
                if split_dma:
                    # Tail tile: ship each 128-row sub-tile as soon as its
                    # evacuation lands, shortening the serial kernel tail.
                    dma_start(
                        y[n0 + a * 128 : n0 + (a + 1) * 128, :].rearrange(
                            "(o p) v -> p o v", p=128
                        ),
                        ysb[:, a, :],
                    )
                elif a == NSUB - 1:
                    dma_start(
                        y[n0 : n0 + TROWS, :].rearrange("(a p) v -> p a v", p=128),
                        ysb[:],
                    )

            Eprev = None
            for i in range(ntiles):
                # Weave phase-1 pairs of tile i between phase-2 sub-tiles of
                # tile i-1 so the PE never waits on exp or PSUM recycling.
                xsT = xsT_ring[i % 3]
                E = Ep.tile([128, KC, TROWS], p2dt)

                def mm1_pair(p):
                    c = 2 * p
                    lh = lhs1h[p // 2]
                    pe = psA.tile([128, 2, TROWS], f32, tag="psA")
                    nc.tensor.matmul(pe[:, 0, :], lh[:, c % 4, :], xsT[:])
                    nc.tensor.matmul(pe[:, 1, :], lh[:, c % 4 + 1, :], xsT[:])
                    nc.scalar.activation(E[:, c : c + 2, :], pe[:], Exp, scale=2.0)

                mm1_pair(0)
                mm1_pair(1)
                if i + 2 < ntiles:
                    phase1_load(i + 2)
                if i + 1 < ntiles:
                    phase1_cast(i + 1)
                if i == 0:
                    vals_dma()
                    mm1_pair(2)
                    mm1_pair(3)
                else:
                    if i == 1:
                        vals_copy()
                    n0 = (i - 1) * TROWS
                    ysb = yp.tile([128, NSUB, D_OUT], f32)
                    for a in range(NSUB):
                        phase2_sub(n0, Eprev, a, ysb, False)
                        if a == 0:
                            mm1_pair(2)
                        elif a == 1:
                            mm1_pair(3)
                Eprev = E

            n0 = (ntiles - 1) * TROWS
            ysb = yp.tile([128, NSUB, D_OUT], f32)
            for a in range(NSUB):
                phase2_sub(n0, Eprev, a, ysb, True)

    nc.compile()
    nc.finalize()
    return nc


def get_nc(use_f32r=USE_F32R, rows=NS, dma="sync", ph2_bf16=True):
    key = ("nc", use_f32r, rows, dma, ph2_bf16)
    if key not in _cache:
        _cache[key] = _build(use_f32r, rows, dma, ph2_bf16)
    return _cache[key]


def make_in_maps(x, ctrs, values, s):
    x = np.ascontiguousarray(x, dtype=np.float32)
    ctrsT = np.ascontiguousarray(
        np.asarray(ctrs, dtype=np.float32).T
    )
    values = np.ascontiguousarray(values, dtype=np.float32)
    s = np.ascontiguousarray(s, dtype=np.float32)
    return [
        {
            "xT": np.ascontiguousarray(x[i * NS : (i + 1) * NS].T),
            "ctrsT": ctrsT,
            "values": values,
            "s": s,
        }
        for i in range(NCORES)
    ]


def run(x, ctrs, values, s, trace=False, use_f32r=USE_F32R, tmpdir=None):
    from concourse.bass_utils import run_bass_kernel_spmd

    nc = get_nc(use_f32r)
    res = run_bass_kernel_spmd(
        nc,
        make_in_maps(x, ctrs, values, s),
        list(range(NCORES)),
        trace=trace,
        tmpdir=tmpdir,
    )
    out = np.concatenate([res.results[i]["y"] for i in range(NCORES)], axis=0)
    return out, res


def kernel(x, ctrs, values, s):
    out, _ = run(x, ctrs, values, s, trace=False)
    return out.astype(np.float32)
